# revision 23
# baseline (speedup 1.0000x reference)
"""Trainium2 Bass kernel for nn_CameraEstimator.

For each batch item b:
    camera[b] = einsum('chw,c->hw', x[b], W)          (C=256 contraction)
    out[b]    = nearest-rotation(camera[b])           (SVD u@vh + det reflection fix)

v2 design:
  * x is pre-converted to fp16 and pre-transposed on the host into the PE
    matmul layout [TPC, 128(ce%128), 18, 128(b-idx)], halving HBM traffic and
    removing all on-device transposes / dtype converts / PSUM copy-backs.
  * Contraction: per b-tile, 18 accumulating fp16 matmuls
    lhsT = x chunk [128(ce), 128(b)], rhs = masked split-fp16 W [128(ce), 18]
    -> PSUM [128(b), 18] = [camera_hi | camera_lo]; camera = hi + lo (one
    GpSimd add reading PSUM directly).
  * SO(3) projection (polar Newton + closed-form reflection fix) runs in
    chunks of tiles so it overlaps the DMA stream; math is table-swap-free:
    Frobenius-scaled Newton (Rsqrt only), Hastings acos polynomial, Taylor
    sin, Newton polish of the smallest eigenvalue.

Sharding: batch split evenly across 8 NeuronCores (data parallel).
"""

import numpy as np

import concourse.bacc as bacc
import concourse.mybir as mybir
from concourse.bass_types import AP
from concourse.tile import TileContext
from concourse import bass_utils

F32 = mybir.dt.float32
F16 = mybir.dt.float16
ALU = mybir.AluOpType
ACT = mybir.ActivationFunctionType
AXL = mybir.AxisListType

B_FULL = 32768
C = 256
E = 9
N_CORES = 8
P = 128
B_LOCAL = B_FULL // N_CORES          # 4096
TPC = B_LOCAL // P                   # 32 matrices per partition
NCH = (C * E) // P                   # 18 chunks of 128 (c,e) pairs

CHUNKS = [20, 12]                    # phase-2 chunk sizes (sum == TPC)
POLAR_PATTERN = "PSP"                # plain / Frobenius-scaled Newton steps

PI = float(np.pi)


def v(base: AP, off: int, *dims) -> AP:
    """Free-dim view of an SBUF tile AP: keep partition dim, set free dims."""
    return AP(base.tensor, base.offset + off,
              [list(base.ap[0])] + [[s, c] for (s, c) in dims])


def make_wm(W: np.ndarray) -> np.ndarray:
    """Masked fp16 W moving operand for the PE contraction.

    wm[j, k, m] = fp16(W[c]) where ce = 128j+k, c = ce//9, if ce%9 == m,
    so that x16_j.T @ wm[j] accumulates camera[b, m] in fp32 PSUM.
    """
    kidx = np.arange(C * E)
    wh = np.zeros((C * E, E), np.float32)
    wh[kidx, kidx % E] = W[kidx // E]
    return np.ascontiguousarray(wh.astype(np.float16).reshape(NCH, P, E))


def make_x16(x: np.ndarray) -> np.ndarray:
    """Host-side fp16 convert + transpose into the PE matmul layout.

    Returns [N_CORES, TPC, 128(p), NCH(j), 128(i)] fp16 where element
    (core, t, p, j, i) = fp16(x[b, c, h, w]) with b = core*B_LOCAL + i*TPC + t
    and flat ce = c*9 + (3h+w) = 128*j + p.
    """
    x16 = x.astype(np.float16)
    xr = x16.reshape(N_CORES, P, TPC, C * E)      # [core, i, t, ce]
    xt = xr.transpose(0, 2, 3, 1)                 # [core, t, ce, i]
    xt = xt.reshape(N_CORES, TPC, NCH, P, P)      # ce -> (j, p)
    xt = xt.transpose(0, 1, 3, 2, 4)              # [core, t, p, j, i]
    return np.ascontiguousarray(xt)


def _project(nc, pjp, cb, cam, y_ap, t0, t1, dcopy_on_act, offload):
    """SO(3) projection for tiles [t0, t1), v4.

    R = polar(G) with G = cam + (1/sigma1) cof(cam): adding (1/s1) cof shifts
    the singular values to (s1 + s2 s3/s1, s2 + sgn(det) s3, sgn(det)(s2 - s3))
    so the polar factor of G is exactly U diag(1,1,sgn det) V^T -- the answer.
    sigma1 comes from the trig closed form for the largest eigenvalue of
    cam^T cam, whose characteristic coefficients are just |cam|_F^2,
    |cof|_F^2 and det^2.  G is also normalized by the analytically-known
    geometric mean of its extreme singular values, so the Newton polar
    iteration needs no per-iteration scale factors for typical samples
    (2 Frobenius-scaled + 2 plain iterations mop up stragglers).

    Plane (per-matrix scalar) chain ops go to GpSimd when `offload` so the
    DVE stays free for other chunks' heavy ops.
    """
    vec = nc.vector
    act = nc.scalar
    pe_ = nc.gpsimd if offload else nc.vector
    T = t1 - t0
    NE = E * T
    f32 = F32

    def mat(X):
        return v(X, 0, (E, T), (3, 3), (1, 3))

    def flat(X):
        return v(X, 0, (1, NE))

    def row0(X):
        return v(X, 0, (E, T), (1, 3))

    def pl(X):
        return v(X, 0, (1, T))

    def bc9(X):
        return v(X, 0, (1, T), (0, E))

    def tile(tag, cols):
        return pjp.tile([P, cols], f32, tag=f"{tag}{T}", name=f"{tag}_{t0}")

    Ya = tile("Ya", NE)
    Yb = tile("Yb", NE)
    Cf = tile("Cf", NE)
    w1_ = tile("w1", NE)
    w2_ = tile("w2", NE)
    D = tile("D", 36 * T)
    td = tile("td", 3 * T)
    _pt = {}

    def p(name):
        if name not in _pt:
            _pt[name] = tile(name, T)
        return _pt[name]

    zb = v(cb(0.0), 0, (0, T))
    pib = v(cb(PI), 0, (0, T))

    def dblock(off):
        return v(D, off, (36, T), (6, 3), (1, 3))

    def cofactor(Y, out):
        src = mat(Y)
        for off in (0, 3, 18, 21):
            vec.tensor_copy(v(D, off, (36, T), (6, 3), (1, 3)), src)
        vec.tensor_tensor(mat(w1_), dblock(7), dblock(14), ALU.mult)
        vec.tensor_tensor(mat(w2_), dblock(8), dblock(13), ALU.mult)
        vec.tensor_tensor(mat(out), mat(w1_), mat(w2_), ALU.subtract)

    def det_of(Y, Cof, out):
        vec.tensor_tensor(v(td, 0, (3, T), (1, 3)), row0(Y), row0(Cof),
                          ALU.mult)
        vec.tensor_reduce(pl(out), v(td, 0, (3, T), (1, 3)), AXL.X, ALU.add)

    # ---- invariants of cam ------------------------------------------------
    cofactor(cam, Cf)
    vec.tensor_tensor(flat(w1_), flat(cam), flat(cam), ALU.mult)
    vec.tensor_reduce(pl(p("uu")), v(w1_, 0, (E, T), (1, E)), AXL.X, ALU.add)
    vec.tensor_tensor(flat(w1_), flat(Cf), flat(Cf), ALU.mult)
    vec.tensor_reduce(pl(p("vv")), v(w1_, 0, (E, T), (1, E)), AXL.X, ALU.add)
    det_of(cam, Cf, p("det"))
    uu, vv, det = p("uu"), p("vv"), p("det")

    # ---- lam1 = largest eigenvalue of cam^T cam (trig closed form) --------
    # cubic s^3 - u s^2 + v s - d2;  q = u/3, p^2 = u^2/9 - v/3
    d2, q, uu9, p26, pp, r_, ar, h, sf, lam = (
        p(n) for n in ("d2", "q", "uu9", "p26", "pp", "r", "ar", "h", "sf",
                       "lam"))
    pe_.tensor_tensor(pl(d2), pl(det), pl(det), ALU.mult)
    pe_.tensor_scalar_mul(pl(q), pl(uu), 1.0 / 3.0)
    pe_.tensor_tensor(pl(uu9), pl(uu), pl(uu), ALU.mult)
    pe_.tensor_scalar_mul(pl(uu9), pl(uu9), 1.0 / 9.0)
    pe_.scalar_tensor_tensor(pl(p26), pl(vv), -1.0 / 3.0, pl(uu9),
                             ALU.mult, ALU.add)
    pe_.tensor_scalar(pl(p26), pl(p26), 0.0, None, ALU.max)
    act.activation(pl(pp), pl(p26), ACT.Sqrt, bias=cb(0.0))
    # detB = (2 uu9 - v) q + d2
    pe_.tensor_scalar(pl(r_), pl(uu9), 2.0, None, ALU.mult)
    pe_.tensor_tensor(pl(r_), pl(r_), pl(vv), ALU.subtract)
    pe_.tensor_tensor(pl(r_), pl(r_), pl(q), ALU.mult)
    pe_.tensor_tensor(pl(r_), pl(r_), pl(d2), ALU.add)
    # r = clamp(detB / (2 p^3), -1, 1)
    pe_.tensor_tensor(pl(h), pl(p26), pl(pp), ALU.mult)
    pe_.tensor_scalar(pl(h), pl(h), 2.0, 1e-30, ALU.mult, ALU.add)
    vec.reciprocal(pl(h), pl(h))
    pe_.tensor_tensor(pl(r_), pl(r_), pl(h), ALU.mult)
    pe_.tensor_scalar(pl(r_), pl(r_), -1.0, 1.0, ALU.max, ALU.min)
    # acos(|r|) = sqrt(1-|r|) * Hastings poly(|r|); odd reflection for r<0
    pe_.tensor_scalar_mul(pl(ar), pl(r_), -1.0)
    pe_.tensor_tensor(pl(ar), pl(ar), pl(r_), ALU.max)
    pe_.tensor_scalar(pl(h), pl(ar), -1.0, 1.0 + 1e-12, ALU.mult, ALU.add)
    act.activation(pl(h), pl(h), ACT.Sqrt, bias=cb(0.0))
    pe_.tensor_scalar(pl(sf), pl(ar), -0.0187293, 0.0742610, ALU.mult,
                      ALU.add)
    pe_.tensor_tensor(pl(sf), pl(sf), pl(ar), ALU.mult)
    pe_.tensor_scalar(pl(sf), pl(sf), -0.2121144, None, ALU.add)
    pe_.tensor_tensor(pl(sf), pl(sf), pl(ar), ALU.mult)
    pe_.tensor_scalar(pl(sf), pl(sf), 1.5707288, None, ALU.add)
    pe_.tensor_tensor(pl(h), pl(sf), pl(h), ALU.mult)        # acos(|r|)
    pe_.tensor_tensor(pl(sf), pl(r_), zb, ALU.is_lt)
    pe_.scalar_tensor_tensor(pl(ar), pl(h), -2.0, pib, ALU.mult, ALU.add)
    pe_.tensor_tensor(pl(ar), pl(ar), pl(sf), ALU.mult)
    pe_.tensor_tensor(pl(h), pl(h), pl(ar), ALU.add)         # acos(r)
    # lam = q + 2 p cos(acos/3)
    pe_.tensor_scalar_mul(pl(h), pl(h), 1.0 / 3.0)
    pe_.tensor_tensor(pl(ar), pl(h), pl(h), ALU.mult)        # th^2
    pe_.tensor_scalar(pl(h), pl(ar), 1.0 / 24.0, -0.5, ALU.mult, ALU.add)
    pe_.tensor_tensor(pl(h), pl(h), pl(ar), ALU.mult)
    pe_.tensor_scalar(pl(h), pl(h), 1.0, None, ALU.add)      # cos(th)
    pe_.tensor_tensor(pl(lam), pl(pp), pl(h), ALU.mult)
    pe_.scalar_tensor_tensor(pl(lam), pl(lam), 2.0, pl(q), ALU.mult, ALU.add)

    # ---- beta = 1/sigma1, prescale G ---------------------------------
    # bet and sig1 share one packed tile so a single Sqrt covers both
    bs2 = tile("bs2", 2 * T)
    bet = v(bs2, 0, (1, T))
    sig1 = v(bs2, T, (1, T))
    s2q, gin = p("s2q"), p("gin")
    vec.reciprocal(bet, pl(lam))
    vec.tensor_copy(sig1, pl(lam))
    act.activation(v(bs2, 0, (1, 2 * T)), v(bs2, 0, (1, 2 * T)), ACT.Sqrt,
                   bias=cb(0.0))
    pe_.tensor_scalar_mul(pl(ar), pl(det), -1.0)
    pe_.tensor_tensor(pl(ar), pl(ar), pl(det), ALU.max)       # |det|
    pe_.tensor_tensor(pl(ar), pl(ar), pl(bet), ALU.mult)
    pe_.tensor_tensor(pl(ar), pl(ar), pl(bet), ALU.mult)
    pe_.tensor_tensor(pl(sig1), pl(sig1), pl(ar), ALU.add)    # s1'
    pe_.tensor_tensor(pl(sig1), pl(sig1), pl(sig1), ALU.mult)
    pe_.tensor_tensor(pl(s2q), pl(uu), pl(lam), ALU.subtract)
    pe_.tensor_tensor(pl(h), pl(bet), pl(det), ALU.mult)
    pe_.scalar_tensor_tensor(pl(s2q), pl(h), 2.0, pl(s2q), ALU.mult, ALU.add)
    pe_.tensor_scalar(pl(s2q), pl(s2q), 1e-20, None, ALU.max)
    pe_.tensor_tensor(pl(s2q), pl(s2q), pl(sig1), ALU.mult)
    act.activation(pl(gin), pl(s2q), ACT.Sqrt, bias=cb(0.0))
    act.activation(pl(gin), pl(gin), ACT.Sqrt, bias=cb(0.0))  # g0
    vec.reciprocal(pl(gin), pl(gin))
    pe_.tensor_tensor(pl(bet), pl(bet), pl(gin), ALU.mult)    # beta/g0
    # G~ = cam/g0 + (beta/g0) cof(cam)
    vec.tensor_tensor(flat(w1_), flat(Cf), bc9(bet), ALU.mult)
    vec.tensor_tensor(flat(w2_), flat(cam), bc9(gin), ALU.mult)
    vec.tensor_tensor(flat(Ya), flat(w1_), flat(w2_), ALU.add)

    # ---- Newton polar iteration on G~ ---------------------------------
    Y = Ya
    sc, muh = p("sc"), p("muh")
    for step in POLAR_PATTERN:
        cofactor(Y, Cf)
        det_of(Y, Cf, det)
        Yn = Yb if Y is Ya else Ya
        if step == "S":
            vec.tensor_tensor(flat(w1_), flat(Y), flat(Y), ALU.mult)
            vec.tensor_reduce(pl(uu), v(w1_, 0, (E, T), (1, E)), AXL.X,
                              ALU.add)
            vec.tensor_tensor(flat(w1_), flat(Cf), flat(Cf), ALU.mult)
            vec.tensor_reduce(pl(vv), v(w1_, 0, (E, T), (1, E)), AXL.X,
                              ALU.add)
            vec.tensor_tensor(pl(sc), pl(det), pl(det), ALU.mult)
            vec.tensor_tensor(pl(sc), pl(uu), pl(sc), ALU.mult)
            vec.reciprocal(pl(sc), pl(sc))
            vec.tensor_tensor(pl(sc), pl(vv), pl(sc), ALU.mult)   # z
            act.activation(pl(muh), pl(sc), ACT.Sqrt, bias=cb(0.0))
            act.activation(pl(muh), pl(muh), ACT.Sqrt, bias=cb(0.0))  # mu
            vec.tensor_tensor(pl(sc), pl(muh), pl(det), ALU.mult)
            vec.reciprocal(pl(sc), pl(sc))
            vec.tensor_scalar_mul(pl(sc), pl(sc), 0.5)     # 0.5/(mu det)
            vec.tensor_scalar_mul(pl(muh), pl(muh), 0.5)   # 0.5 mu
            vec.tensor_tensor(flat(w1_), flat(Y), bc9(muh), ALU.mult)
            vec.tensor_tensor(flat(w2_), flat(Cf), bc9(sc), ALU.mult)
            vec.tensor_tensor(flat(Yn), flat(w1_), flat(w2_), ALU.add)
        else:
            vec.reciprocal(pl(sc), pl(det))
            vec.tensor_scalar_mul(pl(sc), pl(sc), 0.5)
            vec.tensor_tensor(flat(w2_), flat(Cf), bc9(sc), ALU.mult)
            vec.scalar_tensor_tensor(flat(Yn), flat(Y), 0.5, flat(w2_),
                                     ALU.mult, ALU.add)
        Y = Yn

    act.dma_start(out=AP(y_ap.tensor, E * t0, [[E * TPC, P], [1, NE]]),
                  in_=flat(Y))


def _emit(nc, tc, x_ap, wm_ap, y_ap):
    vec = nc.vector
    x_t = x_ap.rearrange("t p j i -> p t (j i)")      # [128, TPC, 2304]

    with tc.tile_pool(name="xin", bufs=TPC + 1) as xpool, \
         tc.tile_pool(name="ps", bufs=4, space="PSUM") as psp, \
         tc.tile_pool(name="pj", bufs=2) as pjp, \
         tc.tile_pool(name="wk", bufs=1) as wp:
        wm_sb = wp.tile([P, NCH * E], F16)
        nc.sync.dma_start(
            out=wm_sb[:],
            in_=AP(wm_ap.tensor, 0, [[E, P], [E * P, NCH], [1, E]]))

        _consts = {}

        def cb(val):
            if val not in _consts:
                ct = wp.tile([P, 1], F32, name=f"const{len(_consts)}")
                vec.memset(ct[:], float(val))
                _consts[val] = ct[:]
            return _consts[val]

        # Warm-up: ramp the PE p-state and preload the Sqrt act table while
        # the DMA stream is still filling, so neither cost lands on the
        # critical chain later.
        warm = wp.tile([P, P], F16)
        vec.memset(warm[:], 0.0)
        nc.scalar.activation(cb(1.0), cb(1.0), ACT.Sqrt, bias=cb(0.0))
        wps = psp.tile([P, 64], F32, tag="warm", name="warmps")
        for wi in range(40):
            nc.tensor.matmul(wps[:], warm[:], warm[:, :64],
                             start=(wi == 0), stop=(wi == 39))

        bounds = []
        t0 = 0
        for T in CHUNKS:
            bounds.append((t0, t0 + T))
            t0 += T

        cams = [wp.tile([P, E * T], F32, name=f"cam{ci}")
                for ci, (t0, T) in enumerate(zip([b[0] for b in bounds],
                                                 CHUNKS))]

        for ci, (t0, t1) in enumerate(bounds):
            cam = cams[ci]
            for t in range(t0, t1):
                xt = xpool.tile([P, C * E], F16, tag="xt", name=f"xt{t}")
                nc.sync.dma_start(out=xt[:], in_=x_t[:, t, :])
                pc = psp.tile([P, E], F32, tag="pc", name=f"pc{t}")
                for j in range(NCH):
                    nc.tensor.matmul(pc[:], xt[:, P * j:P * (j + 1)],
                                     v(wm_sb, E * j, (1, E)),
                                     start=(j == 0), stop=(j == NCH - 1))
                # camera out of PSUM on the Act engine, keeping the DVE
                # queue free for the projection chains
                nc.scalar.copy(v(cam, (t - t0) * E, (1, E)), pc[:])
            last = ci == len(bounds) - 1
            _project(nc, pjp, cb, cam, y_ap, t0, t1,
                     dcopy_on_act=not last, offload=False)


def build():
    nc = bacc.Bacc("TRN2", target_bir_lowering=False, debug=False)
    x = nc.dram_tensor("x16", [TPC, P, NCH, P], F16, kind="ExternalInput")
    wm = nc.dram_tensor("wm", [NCH, P, E], F16, kind="ExternalInput")
    y = nc.dram_tensor("y", [B_LOCAL, 3, 3], F32, kind="ExternalOutput")
    with TileContext(nc) as tc:
        _emit(nc, tc, x.ap(), wm.ap(), y.ap())
    nc.compile()
    return nc


_NC_CACHE = {}


def kernel(x: np.ndarray, W: np.ndarray) -> np.ndarray:
    assert x.shape == (B_FULL, C, 3, 3) and W.shape == (C,)
    if "nc" not in _NC_CACHE:
        _NC_CACHE["nc"] = build()
    nc = _NC_CACHE["nc"]
    x16 = make_x16(np.asarray(x, dtype=np.float32))
    wm = make_wm(np.asarray(W, dtype=np.float32))
    in_maps = [{"x16": x16[i], "wm": wm} for i in range(N_CORES)]
    res = bass_utils.run_bass_kernel_spmd(nc, in_maps,
                                          core_ids=list(range(N_CORES)))
    return np.concatenate([r["y"] for r in res.results], axis=0)


if __name__ == "__main__":
    rng = np.random.default_rng(0)
    x = rng.standard_normal((B_FULL, C, 3, 3), dtype=np.float32)
    W = (rng.standard_normal(C, dtype=np.float32) / np.sqrt(C)).astype(np.float32)
    out = kernel(x=x, W=W)
    print(out.shape, out.dtype)


# revision 25
# speedup vs baseline: 1.0680x; 1.0680x over previous
"""Trainium2 Bass kernel for nn_CameraEstimator.

For each batch item b:
    camera[b] = einsum('chw,c->hw', x[b], W)          (C=256 contraction)
    out[b]    = nearest-rotation(camera[b])           (SVD u@vh + det reflection fix)

v2 design:
  * x is pre-converted to fp16 and pre-transposed on the host into the PE
    matmul layout [TPC, 128(ce%128), 18, 128(b-idx)], halving HBM traffic and
    removing all on-device transposes / dtype converts / PSUM copy-backs.
  * Contraction: per b-tile, 18 accumulating fp16 matmuls
    lhsT = x chunk [128(ce), 128(b)], rhs = masked split-fp16 W [128(ce), 18]
    -> PSUM [128(b), 18] = [camera_hi | camera_lo]; camera = hi + lo (one
    GpSimd add reading PSUM directly).
  * SO(3) projection (polar Newton + closed-form reflection fix) runs in
    chunks of tiles so it overlaps the DMA stream; math is table-swap-free:
    Frobenius-scaled Newton (Rsqrt only), Hastings acos polynomial, Taylor
    sin, Newton polish of the smallest eigenvalue.

Sharding: batch split evenly across 8 NeuronCores (data parallel).
"""

import numpy as np

import concourse.bacc as bacc
import concourse.mybir as mybir
from concourse.bass_types import AP
from concourse.tile import TileContext
from concourse import bass_utils

F32 = mybir.dt.float32
F16 = mybir.dt.float16
ALU = mybir.AluOpType
ACT = mybir.ActivationFunctionType
AXL = mybir.AxisListType

B_FULL = 32768
C = 256
E = 9
N_CORES = 8
P = 128
B_LOCAL = B_FULL // N_CORES          # 4096
TPC = B_LOCAL // P                   # 32 matrices per partition
NCH = (C * E) // P                   # 18 chunks of 128 (c,e) pairs

CHUNKS = [12, 10, 10]                # phase-2 chunk sizes (sum == TPC)
POLAR_PATTERN = "PSP"                # plain / Frobenius-scaled Newton steps

PI = float(np.pi)


def v(base: AP, off: int, *dims) -> AP:
    """Free-dim view of an SBUF tile AP: keep partition dim, set free dims."""
    return AP(base.tensor, base.offset + off,
              [list(base.ap[0])] + [[s, c] for (s, c) in dims])


def make_wm(W: np.ndarray) -> np.ndarray:
    """Masked fp16 W moving operand for the PE contraction.

    wm[j, k, m] = fp16(W[c]) where ce = 128j+k, c = ce//9, if ce%9 == m,
    so that x16_j.T @ wm[j] accumulates camera[b, m] in fp32 PSUM.
    """
    kidx = np.arange(C * E)
    wh = np.zeros((C * E, E), np.float32)
    wh[kidx, kidx % E] = W[kidx // E]
    return np.ascontiguousarray(wh.astype(np.float16).reshape(NCH, P, E))


def make_x16(x: np.ndarray) -> np.ndarray:
    """Host-side fp16 convert + transpose into the PE matmul layout.

    Returns [N_CORES, TPC, 128(p), NCH(j), 128(i)] fp16 where element
    (core, t, p, j, i) = fp16(x[b, c, h, w]) with b = core*B_LOCAL + i*TPC + t
    and flat ce = c*9 + (3h+w) = 128*j + p.
    """
    x16 = x.astype(np.float16)
    xr = x16.reshape(N_CORES, P, TPC, C * E)      # [core, i, t, ce]
    xt = xr.transpose(0, 2, 3, 1)                 # [core, t, ce, i]
    xt = xt.reshape(N_CORES, TPC, NCH, P, P)      # ce -> (j, p)
    xt = xt.transpose(0, 1, 3, 2, 4)              # [core, t, p, j, i]
    return np.ascontiguousarray(xt)


def _project(nc, pjp, cb, cam, y_ap, t0, t1, dcopy_on_act, offload):
    """SO(3) projection for tiles [t0, t1), v4.

    R = polar(G) with G = cam + (1/sigma1) cof(cam): adding (1/s1) cof shifts
    the singular values to (s1 + s2 s3/s1, s2 + sgn(det) s3, sgn(det)(s2 - s3))
    so the polar factor of G is exactly U diag(1,1,sgn det) V^T -- the answer.
    sigma1 comes from the trig closed form for the largest eigenvalue of
    cam^T cam, whose characteristic coefficients are just |cam|_F^2,
    |cof|_F^2 and det^2.  G is also normalized by the analytically-known
    geometric mean of its extreme singular values, so the Newton polar
    iteration needs no per-iteration scale factors for typical samples
    (2 Frobenius-scaled + 2 plain iterations mop up stragglers).

    Plane (per-matrix scalar) chain ops go to GpSimd when `offload` so the
    DVE stays free for other chunks' heavy ops.
    """
    vec = nc.vector
    act = nc.scalar
    pe_ = nc.gpsimd if offload else nc.vector
    T = t1 - t0
    NE = E * T
    f32 = F32

    def mat(X):
        return v(X, 0, (E, T), (3, 3), (1, 3))

    def flat(X):
        return v(X, 0, (1, NE))

    def row0(X):
        return v(X, 0, (E, T), (1, 3))

    def pl(X):
        return v(X, 0, (1, T))

    def bc9(X):
        return v(X, 0, (1, T), (0, E))

    def tile(tag, cols):
        return pjp.tile([P, cols], f32, tag=f"{tag}{T}", name=f"{tag}_{t0}")

    Ya = tile("Ya", NE)
    Yb = tile("Yb", NE)
    Cf = tile("Cf", NE)
    w1_ = tile("w1", NE)
    w2_ = tile("w2", NE)
    D = tile("D", 36 * T)
    td = tile("td", 3 * T)
    _pt = {}

    def p(name):
        if name not in _pt:
            _pt[name] = tile(name, T)
        return _pt[name]

    zb = v(cb(0.0), 0, (0, T))
    pib = v(cb(PI), 0, (0, T))

    def dblock(off):
        return v(D, off, (36, T), (6, 3), (1, 3))

    def cofactor(Y, out):
        src = mat(Y)
        for off in (0, 3, 18, 21):
            vec.tensor_copy(v(D, off, (36, T), (6, 3), (1, 3)), src)
        vec.tensor_tensor(mat(w1_), dblock(7), dblock(14), ALU.mult)
        vec.tensor_tensor(mat(w2_), dblock(8), dblock(13), ALU.mult)
        vec.tensor_tensor(mat(out), mat(w1_), mat(w2_), ALU.subtract)

    def det_of(Y, Cof, out):
        vec.tensor_tensor(v(td, 0, (3, T), (1, 3)), row0(Y), row0(Cof),
                          ALU.mult)
        vec.tensor_reduce(pl(out), v(td, 0, (3, T), (1, 3)), AXL.X, ALU.add)

    # ---- invariants of cam ------------------------------------------------
    cofactor(cam, Cf)
    vec.tensor_tensor(flat(w1_), flat(cam), flat(cam), ALU.mult)
    vec.tensor_reduce(pl(p("uu")), v(w1_, 0, (E, T), (1, E)), AXL.X, ALU.add)
    vec.tensor_tensor(flat(w1_), flat(Cf), flat(Cf), ALU.mult)
    vec.tensor_reduce(pl(p("vv")), v(w1_, 0, (E, T), (1, E)), AXL.X, ALU.add)
    det_of(cam, Cf, p("det"))
    uu, vv, det = p("uu"), p("vv"), p("det")

    # ---- lam1 = largest eigenvalue of cam^T cam (trig closed form) --------
    # cubic s^3 - u s^2 + v s - d2;  q = u/3, p^2 = u^2/9 - v/3
    d2, q, uu9, p26, pp, r_, ar, h, sf, lam = (
        p(n) for n in ("d2", "q", "uu9", "p26", "pp", "r", "ar", "h", "sf",
                       "lam"))
    pe_.tensor_tensor(pl(d2), pl(det), pl(det), ALU.mult)
    pe_.tensor_scalar_mul(pl(q), pl(uu), 1.0 / 3.0)
    pe_.tensor_tensor(pl(uu9), pl(uu), pl(uu), ALU.mult)
    pe_.tensor_scalar_mul(pl(uu9), pl(uu9), 1.0 / 9.0)
    pe_.scalar_tensor_tensor(pl(p26), pl(vv), -1.0 / 3.0, pl(uu9),
                             ALU.mult, ALU.add)
    pe_.tensor_scalar(pl(p26), pl(p26), 0.0, None, ALU.max)
    act.activation(pl(pp), pl(p26), ACT.Sqrt, bias=cb(0.0))
    # detB = (2 uu9 - v) q + d2
    pe_.tensor_scalar(pl(r_), pl(uu9), 2.0, None, ALU.mult)
    pe_.tensor_tensor(pl(r_), pl(r_), pl(vv), ALU.subtract)
    pe_.tensor_tensor(pl(r_), pl(r_), pl(q), ALU.mult)
    pe_.tensor_tensor(pl(r_), pl(r_), pl(d2), ALU.add)
    # r = clamp(detB / (2 p^3), -1, 1)
    pe_.tensor_tensor(pl(h), pl(p26), pl(pp), ALU.mult)
    pe_.tensor_scalar(pl(h), pl(h), 2.0, 1e-30, ALU.mult, ALU.add)
    vec.reciprocal(pl(h), pl(h))
    pe_.tensor_tensor(pl(r_), pl(r_), pl(h), ALU.mult)
    pe_.tensor_scalar(pl(r_), pl(r_), -1.0, 1.0, ALU.max, ALU.min)
    # acos(|r|) = sqrt(1-|r|) * Hastings poly(|r|); odd reflection for r<0
    pe_.tensor_scalar_mul(pl(ar), pl(r_), -1.0)
    pe_.tensor_tensor(pl(ar), pl(ar), pl(r_), ALU.max)
    pe_.tensor_scalar(pl(h), pl(ar), -1.0, 1.0 + 1e-12, ALU.mult, ALU.add)
    act.activation(pl(h), pl(h), ACT.Sqrt, bias=cb(0.0))
    pe_.tensor_scalar(pl(sf), pl(ar), -0.0187293, 0.0742610, ALU.mult,
                      ALU.add)
    pe_.tensor_tensor(pl(sf), pl(sf), pl(ar), ALU.mult)
    pe_.tensor_scalar(pl(sf), pl(sf), -0.2121144, None, ALU.add)
    pe_.tensor_tensor(pl(sf), pl(sf), pl(ar), ALU.mult)
    pe_.tensor_scalar(pl(sf), pl(sf), 1.5707288, None, ALU.add)
    pe_.tensor_tensor(pl(h), pl(sf), pl(h), ALU.mult)        # acos(|r|)
    pe_.tensor_tensor(pl(sf), pl(r_), zb, ALU.is_lt)
    pe_.scalar_tensor_tensor(pl(ar), pl(h), -2.0, pib, ALU.mult, ALU.add)
    pe_.tensor_tensor(pl(ar), pl(ar), pl(sf), ALU.mult)
    pe_.tensor_tensor(pl(h), pl(h), pl(ar), ALU.add)         # acos(r)
    # lam = q + 2 p cos(acos/3)
    pe_.tensor_scalar_mul(pl(h), pl(h), 1.0 / 3.0)
    pe_.tensor_tensor(pl(ar), pl(h), pl(h), ALU.mult)        # th^2
    pe_.tensor_scalar(pl(h), pl(ar), 1.0 / 24.0, -0.5, ALU.mult, ALU.add)
    pe_.tensor_tensor(pl(h), pl(h), pl(ar), ALU.mult)
    pe_.tensor_scalar(pl(h), pl(h), 1.0, None, ALU.add)      # cos(th)
    pe_.tensor_tensor(pl(lam), pl(pp), pl(h), ALU.mult)
    pe_.scalar_tensor_tensor(pl(lam), pl(lam), 2.0, pl(q), ALU.mult, ALU.add)

    # ---- beta = 1/sigma1, prescale G ---------------------------------
    # bet and sig1 share one packed tile so a single Sqrt covers both
    bs2 = tile("bs2", 2 * T)
    bet = v(bs2, 0, (1, T))
    sig1 = v(bs2, T, (1, T))
    s2q, gin = p("s2q"), p("gin")
    vec.reciprocal(bet, pl(lam))
    vec.tensor_copy(sig1, pl(lam))
    act.activation(v(bs2, 0, (1, 2 * T)), v(bs2, 0, (1, 2 * T)), ACT.Sqrt,
                   bias=cb(0.0))
    pe_.tensor_scalar_mul(pl(ar), pl(det), -1.0)
    pe_.tensor_tensor(pl(ar), pl(ar), pl(det), ALU.max)       # |det|
    pe_.tensor_tensor(pl(ar), pl(ar), pl(bet), ALU.mult)
    pe_.tensor_tensor(pl(ar), pl(ar), pl(bet), ALU.mult)
    pe_.tensor_tensor(pl(sig1), pl(sig1), pl(ar), ALU.add)    # s1'
    pe_.tensor_tensor(pl(sig1), pl(sig1), pl(sig1), ALU.mult)
    pe_.tensor_tensor(pl(s2q), pl(uu), pl(lam), ALU.subtract)
    pe_.tensor_tensor(pl(h), pl(bet), pl(det), ALU.mult)
    pe_.scalar_tensor_tensor(pl(s2q), pl(h), 2.0, pl(s2q), ALU.mult, ALU.add)
    pe_.tensor_scalar(pl(s2q), pl(s2q), 1e-20, None, ALU.max)
    pe_.tensor_tensor(pl(s2q), pl(s2q), pl(sig1), ALU.mult)
    act.activation(pl(gin), pl(s2q), ACT.Sqrt, bias=cb(0.0))
    act.activation(pl(gin), pl(gin), ACT.Sqrt, bias=cb(0.0))  # g0
    vec.reciprocal(pl(gin), pl(gin))
    pe_.tensor_tensor(pl(bet), pl(bet), pl(gin), ALU.mult)    # beta/g0
    # G~ = cam/g0 + (beta/g0) cof(cam)
    vec.tensor_tensor(flat(w1_), flat(Cf), bc9(bet), ALU.mult)
    vec.tensor_tensor(flat(w2_), flat(cam), bc9(gin), ALU.mult)
    vec.tensor_tensor(flat(Ya), flat(w1_), flat(w2_), ALU.add)

    # ---- Newton polar iteration on G~ ---------------------------------
    Y = Ya
    sc, muh = p("sc"), p("muh")
    for step in POLAR_PATTERN:
        cofactor(Y, Cf)
        det_of(Y, Cf, det)
        Yn = Yb if Y is Ya else Ya
        if step == "S":
            vec.tensor_tensor(flat(w1_), flat(Y), flat(Y), ALU.mult)
            vec.tensor_reduce(pl(uu), v(w1_, 0, (E, T), (1, E)), AXL.X,
                              ALU.add)
            vec.tensor_tensor(flat(w1_), flat(Cf), flat(Cf), ALU.mult)
            vec.tensor_reduce(pl(vv), v(w1_, 0, (E, T), (1, E)), AXL.X,
                              ALU.add)
            vec.tensor_tensor(pl(sc), pl(det), pl(det), ALU.mult)
            vec.tensor_tensor(pl(sc), pl(uu), pl(sc), ALU.mult)
            vec.reciprocal(pl(sc), pl(sc))
            vec.tensor_tensor(pl(sc), pl(vv), pl(sc), ALU.mult)   # z
            act.activation(pl(muh), pl(sc), ACT.Sqrt, bias=cb(0.0))
            act.activation(pl(muh), pl(muh), ACT.Sqrt, bias=cb(0.0))  # mu
            vec.tensor_tensor(pl(sc), pl(muh), pl(det), ALU.mult)
            vec.reciprocal(pl(sc), pl(sc))
            vec.tensor_scalar_mul(pl(sc), pl(sc), 0.5)     # 0.5/(mu det)
            vec.tensor_scalar_mul(pl(muh), pl(muh), 0.5)   # 0.5 mu
            vec.tensor_tensor(flat(w1_), flat(Y), bc9(muh), ALU.mult)
            vec.tensor_tensor(flat(w2_), flat(Cf), bc9(sc), ALU.mult)
            vec.tensor_tensor(flat(Yn), flat(w1_), flat(w2_), ALU.add)
        else:
            vec.reciprocal(pl(sc), pl(det))
            vec.tensor_scalar_mul(pl(sc), pl(sc), 0.5)
            vec.tensor_tensor(flat(w2_), flat(Cf), bc9(sc), ALU.mult)
            vec.scalar_tensor_tensor(flat(Yn), flat(Y), 0.5, flat(w2_),
                                     ALU.mult, ALU.add)
        Y = Yn

    act.dma_start(out=AP(y_ap.tensor, E * t0, [[E * TPC, P], [1, NE]]),
                  in_=flat(Y))


def _emit(nc, tc, x_ap, wm_ap, y_ap):
    vec = nc.vector
    x_t = x_ap.rearrange("t p j i -> p t (j i)")      # [128, TPC, 2304]

    with tc.tile_pool(name="xin", bufs=TPC + 1) as xpool, \
         tc.tile_pool(name="ps", bufs=4, space="PSUM") as psp, \
         tc.tile_pool(name="pj", bufs=2) as pjp, \
         tc.tile_pool(name="wk", bufs=1) as wp:
        wm_sb = wp.tile([P, NCH * E], F16)
        nc.sync.dma_start(
            out=wm_sb[:],
            in_=AP(wm_ap.tensor, 0, [[E, P], [E * P, NCH], [1, E]]))

        _consts = {}

        def cb(val):
            if val not in _consts:
                ct = wp.tile([P, 1], F32, name=f"const{len(_consts)}")
                vec.memset(ct[:], float(val))
                _consts[val] = ct[:]
            return _consts[val]

        # Preload the Sqrt act table during the DMA ramp so the 1.3us table
        # load doesn't land on the first projection chain.
        nc.scalar.activation(cb(1.0), cb(1.0), ACT.Sqrt, bias=cb(0.0))

        bounds = []
        t0 = 0
        for T in CHUNKS:
            bounds.append((t0, t0 + T))
            t0 += T

        cams = [wp.tile([P, E * T], F32, name=f"cam{ci}")
                for ci, (t0, T) in enumerate(zip([b[0] for b in bounds],
                                                 CHUNKS))]

        for ci, (t0, t1) in enumerate(bounds):
            cam = cams[ci]
            for t in range(t0, t1):
                xt = xpool.tile([P, C * E], F16, tag="xt", name=f"xt{t}")
                nc.sync.dma_start(out=xt[:], in_=x_t[:, t, :])
                pc = psp.tile([P, E], F32, tag="pc", name=f"pc{t}")
                for j in range(NCH):
                    nc.tensor.matmul(pc[:], xt[:, P * j:P * (j + 1)],
                                     v(wm_sb, E * j, (1, E)),
                                     start=(j == 0), stop=(j == NCH - 1))
                # camera out of PSUM on the Act engine, keeping the DVE
                # queue free for the projection chains
                nc.scalar.copy(v(cam, (t - t0) * E, (1, E)), pc[:])
            last = ci == len(bounds) - 1
            _project(nc, pjp, cb, cam, y_ap, t0, t1,
                     dcopy_on_act=not last, offload=False)


def build():
    nc = bacc.Bacc("TRN2", target_bir_lowering=False, debug=False)
    x = nc.dram_tensor("x16", [TPC, P, NCH, P], F16, kind="ExternalInput")
    wm = nc.dram_tensor("wm", [NCH, P, E], F16, kind="ExternalInput")
    y = nc.dram_tensor("y", [B_LOCAL, 3, 3], F32, kind="ExternalOutput")
    with TileContext(nc) as tc:
        _emit(nc, tc, x.ap(), wm.ap(), y.ap())
    nc.compile()
    return nc


_NC_CACHE = {}


def kernel(x: np.ndarray, W: np.ndarray) -> np.ndarray:
    assert x.shape == (B_FULL, C, 3, 3) and W.shape == (C,)
    if "nc" not in _NC_CACHE:
        _NC_CACHE["nc"] = build()
    nc = _NC_CACHE["nc"]
    x16 = make_x16(np.asarray(x, dtype=np.float32))
    wm = make_wm(np.asarray(W, dtype=np.float32))
    in_maps = [{"x16": x16[i], "wm": wm} for i in range(N_CORES)]
    res = bass_utils.run_bass_kernel_spmd(nc, in_maps,
                                          core_ids=list(range(N_CORES)))
    return np.concatenate([r["y"] for r in res.results], axis=0)


if __name__ == "__main__":
    rng = np.random.default_rng(0)
    x = rng.standard_normal((B_FULL, C, 3, 3), dtype=np.float32)
    W = (rng.standard_normal(C, dtype=np.float32) / np.sqrt(C)).astype(np.float32)
    out = kernel(x=x, W=W)
    print(out.shape, out.dtype)
